# revision 4
# baseline (speedup 1.0000x reference)
"""ColumnParallelLinearWithDelta: GPTQ-int4 LoRA-delta matmul on 8 trn2 cores.

out[d] = x @ dequant(qweight[d], qzeros[d], scales[d]) + x @ base_weight.T

Sharding: column-parallel — out_features (4096) split into 8 slices of 512,
one per NeuronCore; x replicated. Each core computes its [8, 256, 512] slice.

Math (per core, out-col slice ns):
  W[k, n]  = s[g(k), n] * (w4[k, n] - (z4[g(k), n] + 1)),  g(k) = k // 128
  delta    = x @ W = x @ (s .* w4)  -  xs @ (s .* (z4 + 1))
  with xs[t, g] = sum_{k in g} x[t, k]   (host-precomputed group sums)
  out[d]   = delta_d + base,  base = x @ base_weight[ns, :].T

Device pipeline per (adapter d, row-chunk rc of 128 packed int32 rows):
  - DMA packed qweight chunk as int16 [128, 1024]
  - 4x tensor_scalar (>> 4*sh) & 0xF  -> fp16 nibble planes (int16 h = 2n+e
    holds nibbles j = 4e + sh, i.e. plane sh contains j=sh (even h) and
    j=sh+4 (odd h) interleaved)
  - 4x tensor_tensor multiply by scale tile s2 (partition-replicated, x2
    free-interleaved; host-prepped) -> scaled fp16 weights
  - 16 matmuls: stationary xT tile for sub-chunk (rc, j) x moving nibble
    plane (stride-2 free AP), accumulating in PSUM f32
Then per adapter: K=32 correction matmul (-xs^T x sz) and an identity
matmul adding the shared base output; ScalarE copies PSUM->SBUF, DMA out.
"""

import numpy as np

# ---- problem constants (hardcoded; kernel.py must be self-contained) ----
T = 256          # tokens
IN = 4096        # in_features
OUT = 4096       # out_features
D = 8            # adapters
GROUP = 128      # quant group size
G = IN // GROUP  # 32 groups
NCORES = 8
NC_OUT = OUT // NCORES   # 512 out cols per core
RC = 4                   # row chunks of 128 packed int32 rows (512 rows total)

_PROGRAM_CACHE: dict = {}


def _build_program():
    import concourse.bacc as bacc
    import concourse.mybir as mybir
    import concourse.tile as tile
    from concourse.masks import make_identity

    nc = bacc.Bacc("TRN2", target_bir_lowering=False, debug=False)

    fp16 = mybir.dt.float16
    d_xt = nc.dram_tensor("xt", (128, RC * 8 * T), fp16, kind="ExternalInput")
    d_negxs = nc.dram_tensor("negxs", (G, T), fp16, kind="ExternalInput")
    d_qw16 = nc.dram_tensor(
        "qw16", (D, RC, 128, 1024), mybir.dt.int16, kind="ExternalInput"
    )
    d_s2 = nc.dram_tensor("s2", (D, RC, 128, 1024), fp16, kind="ExternalInput")
    d_wb = nc.dram_tensor("wb", (RC, 128, 8 * NC_OUT), fp16, kind="ExternalInput")
    d_sz = nc.dram_tensor("sz", (G, D * NC_OUT), fp16, kind="ExternalInput")
    d_out = nc.dram_tensor("out", (D, T, NC_OUT), mybir.dt.float32,
                           kind="ExternalOutput")

    AT = mybir.AluOpType

    with tile.TileContext(nc) as tc:
        with (
            tc.tile_pool(name="const", bufs=1) as cpool,
            tc.tile_pool(name="qw", bufs=3) as qpool,
            tc.tile_pool(name="s2", bufs=3) as spool,
            tc.tile_pool(name="v", bufs=2) as vpool,
            tc.tile_pool(name="wb", bufs=2) as wpool,
            tc.tile_pool(name="outp", bufs=4) as opool,
            tc.tile_pool(name="ps", bufs=2, space="PSUM") as ppool,
            tc.tile_pool(name="psb", bufs=1, space="PSUM") as pbpool,
        ):
            xt_sb = cpool.tile([128, RC * 8 * T], fp16)
            negxs_sb = cpool.tile([G, T], fp16)
            sz_sb = cpool.tile([G, D * NC_OUT], fp16)
            base_sb = cpool.tile([128, 2 * NC_OUT], fp16)
            ident = cpool.tile([128, 128], fp16)

            nc.sync.dma_start(xt_sb[:], d_xt[:])
            nc.sync.dma_start(negxs_sb[:], d_negxs[:])
            nc.sync.dma_start(sz_sb[:], d_sz[:])
            make_identity(nc, ident[:])

            def xt_tile(rc, j, th):
                off = (rc * 8 + j) * T + th * 128
                return xt_sb[:, off:off + 128]

            # ---- base: x @ base_weight[ns, :].T ----
            ps_b = [pbpool.tile([128, NC_OUT], mybir.dt.float32, tag=f"psb{t}",
                                name=f"psb{t}") for t in range(2)]
            for rc in range(RC):
                wb_t = wpool.tile([128, 8 * NC_OUT], fp16)
                nc.sync.dma_start(wb_t[:], d_wb[rc, :, :])
                for j in range(8):
                    rhs = wb_t[:, j * NC_OUT:(j + 1) * NC_OUT]
                    for th in range(2):
                        nc.tensor.matmul(
                            ps_b[th][:],
                            lhsT=xt_tile(rc, j, th),
                            rhs=rhs,
                            start=(rc == 0 and j == 0),
                            stop=(rc == RC - 1 and j == 7),
                        )
            for th in range(2):
                nc.scalar.copy(base_sb[:, th * NC_OUT:(th + 1) * NC_OUT],
                               ps_b[th][:])

            # ---- adapters ----
            for d in range(D):
                ps = [ppool.tile([128, NC_OUT], mybir.dt.float32, tag=f"ps{t}",
                                 name=f"ps{t}") for t in range(2)]
                for rc in range(RC):
                    qw_t = qpool.tile([128, 1024], mybir.dt.int16)
                    nc.sync.dma_start(qw_t[:], d_qw16[d, rc, :, :])
                    s2_t = spool.tile([128, 1024], fp16)
                    nc.sync.dma_start(s2_t[:], d_s2[d, rc, :, :])
                    v_tiles = []
                    for sh in range(4):
                        vr = vpool.tile([128, 1024], mybir.dt.int16,
                                        tag=f"vr{sh}", name=f"vr{sh}")
                        nc.vector.tensor_scalar(
                            out=vr[:], in0=qw_t[:],
                            scalar1=4 * sh, scalar2=0xF,
                            op0=AT.logical_shift_right, op1=AT.bitwise_and,
                        )
                        v = vpool.tile([128, 1024], fp16, tag=f"v{sh}", name=f"v{sh}")
                        nc.vector.tensor_tensor(
                            out=v[:], in0=vr[:], in1=s2_t[:], op=AT.mult
                        )
                        v_tiles.append(v)
                    for j in range(8):
                        sh, e = j % 4, j // 4
                        rhs = v_tiles[sh][:, e::2]
                        for th in range(2):
                            nc.tensor.matmul(
                                ps[th][:],
                                lhsT=xt_tile(rc, j, th),
                                rhs=rhs,
                                start=(rc == 0 and j == 0),
                                stop=False,
                            )
                # zeros correction (K=32) + base add (identity matmul)
                for th in range(2):
                    nc.tensor.matmul(
                        ps[th][:],
                        lhsT=negxs_sb[:, th * 128:(th + 1) * 128],
                        rhs=sz_sb[:, d * NC_OUT:(d + 1) * NC_OUT],
                        start=False, stop=False,
                    )
                    nc.tensor.matmul(
                        ps[th][:],
                        lhsT=ident[:],
                        rhs=base_sb[:, th * NC_OUT:(th + 1) * NC_OUT],
                        start=False, stop=True,
                    )
                for th in range(2):
                    o_t = opool.tile([128, NC_OUT], mybir.dt.float32)
                    nc.scalar.copy(o_t[:], ps[th][:])
                    nc.sync.dma_start(
                        d_out[d, th * 128:(th + 1) * 128, :], o_t[:]
                    )

    nc.compile()
    return nc


def _prep_inputs(x, base_weight, qweight, qzeros, scales):
    """Host-side layout prep. Returns list of 8 per-core input maps."""
    x = np.asarray(x, dtype=np.float32)
    base_weight = np.asarray(base_weight, dtype=np.float32)
    qweight = np.asarray(qweight, dtype=np.int32)
    qzeros = np.asarray(qzeros, dtype=np.int32)
    scales = np.asarray(scales, dtype=np.float32)

    # stationary x tiles: xt[p, (rc*8+j)*T + t] = x[t, 8*(128*rc+p)+j]
    xr = np.ascontiguousarray(x.T).reshape(RC, 128, 8, T)        # [rc, p, j, t]
    xt = np.ascontiguousarray(xr.transpose(1, 0, 2, 3)).reshape(128, RC * 8 * T)
    xt = xt.astype(np.float16)

    # group sums of x (for the zeros-correction contraction), negated
    xs = x.reshape(T, G, GROUP).sum(axis=2)                       # [t, g]
    negxs = np.ascontiguousarray((-xs.T)).astype(np.float16)      # [g, t]

    # unpack qzeros (packed along out cols): z4[d, g, 8m+jj]
    jj = 4 * np.arange(8, dtype=np.int32)
    z4 = ((qzeros[:, :, :, None] >> jj[None, None, None, :]) & 0xF)
    z4 = z4.reshape(D, G, OUT)                                    # [d, g, n]
    sz_full = scales * (z4 + 1).astype(np.float32)                # [d, g, n]

    in_maps = []
    for c in range(NCORES):
        ns = slice(c * NC_OUT, (c + 1) * NC_OUT)

        qw_c = np.ascontiguousarray(qweight[:, :, ns])            # [D, 512, 512]
        qw16 = qw_c.reshape(D, RC, 128, NC_OUT).view(np.int16)    # [D,RC,128,1024]

        s_c = scales[:, :, ns]                                    # [D, G, 512]
        s2 = s_c.reshape(D, RC, 8, NC_OUT)                        # [d, rc, gg, n]
        s2 = np.repeat(s2, 16, axis=2)                            # [d, rc, 128, n]
        s2 = np.repeat(s2, 2, axis=3).astype(np.float16)          # [d,rc,128,1024]

        bw_c = base_weight[ns, :]                                 # [512, 4096]
        wb = np.ascontiguousarray(bw_c.T).reshape(RC, 128, 8, NC_OUT)
        wb = wb.reshape(RC, 128, 8 * NC_OUT).astype(np.float16)

        sz_c = sz_full[:, :, ns]                                  # [D, G, 512]
        sz = np.ascontiguousarray(sz_c.transpose(1, 0, 2)).reshape(G, D * NC_OUT)
        sz = sz.astype(np.float16)

        in_maps.append({
            "xt": xt, "negxs": negxs,
            "qw16": np.ascontiguousarray(qw16),
            "s2": np.ascontiguousarray(s2),
            "wb": np.ascontiguousarray(wb),
            "sz": sz,
        })
    return in_maps


def _run(in_maps, trace=False):
    from concourse import bass_utils
    if "nc" not in _PROGRAM_CACHE:
        _PROGRAM_CACHE["nc"] = _build_program()
    nc = _PROGRAM_CACHE["nc"]
    res = bass_utils.run_bass_kernel_spmd(
        nc, in_maps, core_ids=list(range(NCORES)), trace=trace
    )
    return res


def kernel(x, base_weight, qweight, qzeros, scales, g_idx, _trace=False,
           _return_results=False):
    in_maps = _prep_inputs(x, base_weight, qweight, qzeros, scales)
    res = _run(in_maps, trace=_trace)
    out = np.concatenate([res.results[c]["out"] for c in range(NCORES)], axis=2)
    if _return_results:
        return out, res
    return out
